# revision 41
# baseline (speedup 1.0000x reference)
"""Trainium2 Bass kernel for Dynamic ReLU-B (nn_Dynamic_Relu_B_70291434766473).

Reference computation (per sample n, channel c, pixel p):
    pooled[n,c] = mean_p x[n,c,p]
    h = relu(pooled @ fc1_w.T + fc1_b)                       # [N, 32]
    delta = 2*sigmoid(einsum('koh,nh->kno', fc2_w, h) + fc2_b) - 1
    alpha = delta[..., 0::2]; beta = delta[..., 1::2]        # [K, N, C]
    a = [1,0][k] + 1.0*alpha ; b = [1,0][k] + 0.5*beta
    out = max_k (x * a[k] + b[k])

Strategy: pure data parallel over batch N=32 across 8 NeuronCores (4
samples/core).  x and out move through HBM as bf16 (host casts), which
halves DMA traffic to ~12.9 MB/core and unlocks the DVE 16-bit perf
modes (4x tensor_scalar, 2x tensor_tensor).  End-to-end bf16 error is
~5e-3 vs the 2e-2 gate.

  - pooling runs on the otherwise-idle PE via linearity:
    fc1_pre = W1 @ (sum_p x) = sum_p (W1 @ x).  Per sample, 8 pixel
    chunks of 392 go through matmuls with the [128, 32] fc1 weights,
    landing in 4 partition-bands x 2 column-halves of one [128, 1024]
    (2-bank) PSUM tile; a single accumulate pass per column-half (one
    on ACT, one on DVE) gives band partials, and a tiny [128, 32]
    summing selector matmul folds the 4 bands and lands fc1-preact on
    partitions 0..31.  relu(scale=1/HW, bias=fc1b) -> ht column.
  - fc2 is computed transposed: the [33, 128] w2r chunks are the
    stationary operand and the tiny ht column the moving one, so
    z lands per-channel on partitions directly ([128, 8] PSUM);
    2*sigmoid(z)-1 = tanh(z/2) is one [128, 8] ACT op.  No PE
    transposes needed.
  - apply: branch k=1 on ScalarE activation(Identity, scale=a1,
    bias=b1) in pixel-halves (keeps the ACT queue responsive for the
    tiny head ops), late tiles on VectorE; branch k=0 and the max on
    VectorE (bf16 4x / 2x modes); params stay fp32 per-partition
    scalars (exempt from the 16-bit perf-mode dtype rule).
  - DMA: x loads ride ch0->SP ring, ch1->ACT ring (parallel ramp),
    all stores ride the SP ring behind the ch0 loads; the last
    sample's stores split across both rings.  All constants ride ONE
    packed SWDGE (gpsimd) transfer (SWDGE serializes at ~2us/DMA).
    First two samples load in pixel-halves, last two apply in
    pixel-halves (shorter ramp/tail).  Order-only deps keep the tiny
    per-sample head ops (presum/zT on PE, tanh/ab on ACT) ahead of
    streaming ops in each engine's static queue — the Tile scheduler's
    optimistic DMA model otherwise front-loads the next sample's G
    matmuls, which stall on real loads and head-of-line block the
    previous head.
"""

import numpy as np

N, C, H, W = 32, 256, 56, 56
HW = H * W
HID = C // 8  # 32
NCORES = 8
NPC = N // NCORES  # samples per core
NCH = 8            # pixel chunks for the PE pooling matmuls
CHK = HW // NCH    # 392

_CACHE = {}


def _build_program():
    """Build (and cache) the compiled Bass program for one core."""
    if "nc" in _CACHE:
        return _CACHE["nc"]

    import concourse.bacc as bacc
    import concourse.mybir as mybir
    import concourse.tile as tile

    f32 = mybir.dt.float32
    bf16 = mybir.dt.bfloat16
    AF = mybir.ActivationFunctionType
    ALU = mybir.AluOpType

    nc = bacc.Bacc(
        "TRN2",
        target_bir_lowering=False,
        debug=False,
        enable_asserts=True,
        num_devices=NCORES,
    )

    xs = nc.dram_tensor("xs", [NPC, C, HW], bf16, kind="ExternalInput").ap()
    # all constants packed into ONE bf16 tensor: cols [0:32] w1tp ch0,
    # [32:64] w1tp ch1, [64:96] sump, [96] fc1b, [128:1152] w2r
    # (rows 0..32).  One SWDGE DMA instead of five — SWDGE serializes
    # at ~2us per transfer, which starved the first head in v1.
    cpk = nc.dram_tensor("cpk", [128, 128 + 8 * 128], bf16,
                         kind="ExternalInput").ap()
    out = nc.dram_tensor("out", [NPC, C, HW], bf16, kind="ExternalOutput").ap()

    ring = {0: nc.sync, 1: nc.scalar}  # per-channel-half HWDGE ring

    HH = HW // 2
    SPLIT_LOAD = (0, 1)   # samples loaded in pixel-halves
    SPLIT_APPLY = (2, 3)  # samples whose apply runs in pixel-halves
    # y1 tiles on ACT (rest on DVE); sample 3 splits across engines.
    # (Pool/gpsimd TensorTensor and TensorScalar are rejected by walrus
    # codegen — "Instruction engine check failed (Pool)" — so the Pool
    # engine can only do memset here.)
    ACT_Y1 = {(0, 0), (0, 1), (1, 0), (1, 1), (3, 0)}
    POOL_MAX = set()

    with tile.TileContext(nc) as tc:
        with (
            tc.tile_pool(name="const", bufs=1) as cpool,
            tc.tile_pool(name="x", bufs=2 * NPC) as xpool,
            tc.tile_pool(name="y1", bufs=4) as ypool,
            tc.tile_pool(name="o", bufs=6) as opool,
            tc.tile_pool(name="rd", bufs=2) as rpool,
            tc.tile_pool(name="small", bufs=1) as smpool,
            tc.tile_pool(name="ps", bufs=2, space="PSUM") as pspool,
        ):
            # --- constants: one packed SWDGE transfer off the HWDGE rings ---
            cpk_t = cpool.tile([128, 128 + 8 * 128], bf16, tag="cpk")
            nc.gpsimd.dma_start(cpk_t[:], cpk[:])
            w1tp_t = [cpk_t[:, 0:HID], cpk_t[:, HID:2 * HID]]
            sump_t = cpk_t[:, 2 * HID:3 * HID]
            w2r_t = cpk_t[0:HID + 1, 128:128 + 8 * 128]
            # fc1 bias needs f32 to pair with the f32 psum input of the
            # relu; one-time tiny cast on DVE
            fc1b_t = cpool.tile([HID, 1], f32, tag="fc1b")
            nc.vector.tensor_copy(fc1b_t[:], cpk_t[0:HID, 3 * HID:3 * HID + 1])

            # ht holds all samples' hidden vectors; row HID is the ones row
            # for the fc2 bias trick (memset on the idle Pool engine).
            ht = smpool.tile([HID + 1, NPC], bf16, tag="ht")
            nc.gpsimd.memset(ht[HID:HID + 1, :], 1.0)

            # --- load all x tiles (ch0 -> sync ring, ch1 -> scalar ring) ---
            xt = {}
            last_disp = {0: None, 1: None}
            for n in range(NPC):
                for ch in range(2):
                    t = xpool.tile([128, HW], bf16, tag="x")
                    if n in SPLIT_LOAD:
                        for h in range(2):
                            last_disp[ch] = ring[ch].dma_start(
                                t[:, h * HH:(h + 1) * HH],
                                xs[n, ch * 128:(ch + 1) * 128,
                                   h * HH:(h + 1) * HH],
                            )
                    else:
                        last_disp[ch] = ring[ch].dma_start(
                            t[:], xs[n, ch * 128:(ch + 1) * 128, :]
                        )
                    xt[(n, ch)] = t

            tts, abs_, tanh_insts = {}, {}, {}
            ab_last, rb_insts, zt_last = {}, {}, {}

            def head(s):
                # G = W1 @ x via PE: chunk j -> band j%4, col-half j//4.
                # [128, 1024] = exactly 2 PSUM banks; col-half 1 starts at
                # col 512 so no matmul crosses a bank boundary.
                g = pspool.tile([128, 1024], f32, tag="g")
                for j in range(NCH):
                    band = j % 4
                    col = (j // 4) * 512
                    for ch in range(2):
                        mm = nc.tensor.matmul(
                            g[32 * band:32 * band + 32, col:col + CHK],
                            w1tp_t[ch],
                            xt[(s, ch)][:, j * CHK:(j + 1) * CHK],
                            start=(ch == 0), stop=(ch == 1),
                            tile_position=(0, 32 * band),
                        )
                        if j == 0 and ch == 0 and s - 1 in zt_last:
                            # the scheduler's DMA model is optimistic: it
                            # front-loads G(s) into the PE queue where it
                            # stalls on real loads, head-of-line blocking
                            # the previous head's tiny presum/zT matmuls.
                            tile.add_dep_helper(
                                mm.ins, zt_last[s - 1].ins, sync=False,
                                reason="prev head zT before next G",
                            )
                # reduce each column-half over pixels -> band partials
                # (bf16 partials so they can feed the bf16 selector matmul)
                pre2 = smpool.tile([128, 2], bf16, tag=f"pre{s}")
                dA = rpool.tile([128, CHK], f32, tag="dA")
                dB = rpool.tile([128, CHK], f32, tag="dB")
                with nc.allow_low_precision(reason="bf16 band partials"):
                    ra = nc.scalar.activation(
                        dA[:], g[:, 0:CHK], AF.Copy, accum_out=pre2[:, 0:1],
                    )
                    if s >= 2:
                        # late samples: the DVE queue is backed up with
                        # apply ops by now, so a DVE-side reduce would
                        # delay presum/relu (observed 2.1-2.6us ACT idle);
                        # run it on ACT right behind rA instead
                        nc.scalar.activation(
                            dB[:], g[:, 512:512 + CHK], AF.Copy,
                            accum_out=pre2[:, 1:2],
                        )
                    else:
                        rb_insts[s] = nc.vector.tensor_scalar(
                            dB[:], g[:, 512:512 + CHK], 1.0, None,
                            ALU.mult, ALU.add, accum_out=pre2[:, 1:2],
                        )
                if s == 0:
                    # all ch1 load dispatches (which share the ACT engine
                    # queue) must come before any ACT compute
                    tile.add_dep_helper(
                        ra.ins, last_disp[1].ins, sync=False,
                        reason="load dispatches before ACT compute",
                    )
                # band-sum + partition shift onto 0..31 via selector matmul
                php = pspool.tile([HID, 1], f32, tag="php")
                nc.tensor.matmul(
                    php[:], sump_t, pre2[:, 0:1], start=True, stop=False,
                )
                nc.tensor.matmul(
                    php[:], sump_t, pre2[:, 1:2], start=False, stop=True,
                )
                # fc1: relu(php/HW + fc1b) -> ht column s
                nc.scalar.activation(
                    ht[0:HID, s:s + 1], php[:],
                    AF.Relu, bias=fc1b_t[:], scale=1.0 / HW,
                )
                # fc2 transposed: z lands per-channel on partitions
                pzt = pspool.tile([128, 8], f32, tag="pzt")
                for cb in range(8):
                    zt_last[s] = nc.tensor.matmul(
                        pzt[:, cb:cb + 1],
                        cpk_t[0:HID + 1, 128 + cb * 128:128 + (cb + 1) * 128],
                        ht[:, s:s + 1],
                        start=True, stop=True,
                    )
                # t = tanh(z/2) = 2*sigmoid(z) - 1
                tt = smpool.tile([128, 8], f32, tag=f"tt{s}")
                tanh_insts[s] = nc.scalar.activation(
                    tt[:], pzt[:], AF.Tanh, bias=0.0, scale=0.5
                )
                tts[s] = tt
                # param transforms on ACT right behind the tanh (tiny; on
                # DVE they head-of-line block behind streaming apply ops):
                #   a0 = 1 + t   b0 = 0.5*t + 1   a1 = t   b1 = 0.5*t
                ab = smpool.tile([128, 8], f32, tag=f"ab{s}")
                nc.scalar.activation(
                    ab[:, 0:2], tt[:, 0:2], AF.Identity, bias=1.0, scale=1.0
                )
                nc.scalar.activation(
                    ab[:, 2:4], tt[:, 2:4], AF.Identity, bias=1.0, scale=0.5
                )
                ab_last[s] = nc.scalar.activation(
                    ab[:, 6:8], tt[:, 6:8], AF.Identity, bias=0.0, scale=0.5
                )
                abs_[s] = ab

            def apply_s(s):
                # emit all y1s, then all y0s, then the maxes, so neither
                # engine's queue head-of-line blocks on the other engine.
                tt, ab = tts[s], abs_[s]
                units = []  # (ch, pixel-slice) at store granularity
                for ch in range(2):
                    if s in SPLIT_APPLY:
                        units.append((ch, slice(0, HH)))
                        units.append((ch, slice(HH, HW)))
                    else:
                        units.append((ch, slice(0, HW)))
                y1s, os_ = {}, {}
                for u, (ch, sl) in enumerate(units):
                    y1 = ypool.tile([128, sl.stop - sl.start], bf16, tag="y1")
                    if (s, ch) in ACT_Y1:
                        # halves keep the ACT queue responsive for head ops
                        m = (sl.stop - sl.start) // 2
                        for hs in (slice(0, m), slice(m, sl.stop - sl.start)):
                            inst = nc.scalar.activation(
                                y1[:, hs],
                                xt[(s, ch)][:, sl.start + hs.start:
                                            sl.start + hs.stop],
                                AF.Identity,
                                bias=ab[:, 6 + ch:7 + ch],
                                scale=tt[:, 4 + ch:5 + ch],
                            )
                            for s_later in ab_last:
                                if s_later > s:
                                    # keep ALL later heads' tanh/ab chains
                                    # ahead of these streaming ops on ACT
                                    tile.add_dep_helper(
                                        inst.ins, ab_last[s_later].ins,
                                        sync=False,
                                        reason="head chain before y1",
                                    )
                    else:
                        nc.vector.tensor_scalar(
                            y1[:], xt[(s, ch)][:, sl],
                            tt[:, 4 + ch:5 + ch], ab[:, 6 + ch:7 + ch],
                            ALU.mult, ALU.add,
                        )
                    y1s[u] = y1
                for u, (ch, sl) in enumerate(units):
                    o = opool.tile([128, sl.stop - sl.start], bf16, tag="o")
                    nc.vector.tensor_scalar(
                        o[:], xt[(s, ch)][:, sl],
                        ab[:, ch:ch + 1], ab[:, 2 + ch:3 + ch],
                        ALU.mult, ALU.add,
                    )
                    os_[u] = o
                for u, (ch, sl) in enumerate(units):
                    o, y1 = os_[u], y1s[u]
                    if (s, ch) in POOL_MAX:
                        nc.gpsimd.tensor_tensor(o[:], o[:], y1[:], ALU.max)
                    else:
                        nc.vector.tensor_max(o[:], o[:], y1[:])
                    if s == NPC - 1:
                        # final sample: split each store across both rings
                        # so the tail transfers drain in parallel
                        m = (sl.stop - sl.start) // 2
                        ring[0].dma_start(
                            out[s, ch * 128:(ch + 1) * 128,
                                sl.start:sl.start + m], o[:, 0:m],
                        )
                        ring[1].dma_start(
                            out[s, ch * 128:(ch + 1) * 128,
                                sl.start + m:sl.stop], o[:, m:],
                        )
                    else:
                        ring[0].dma_start(
                            out[s, ch * 128:(ch + 1) * 128, sl], o[:]
                        )

            # heads lead (they pace on DMA arrival); each sample's apply
            # streams behind the NEXT head so param latency never hides
            # behind streaming ops.
            head(0)
            head(1)
            head(2)
            apply_s(0)
            head(3)
            apply_s(1)
            apply_s(2)
            apply_s(3)

    nc.compile()
    _CACHE["nc"] = nc
    return nc


def make_inputs(x, fc1_w, fc1_b, fc2_w, fc2_b):
    """Host-side prep: shard x, rearrange weights into device layouts."""
    import ml_dtypes

    bf16 = ml_dtypes.bfloat16
    x = np.ascontiguousarray(
        np.asarray(x, dtype=np.float32).reshape(N, C, HW)
    ).astype(bf16)
    # one packed bf16 const tensor: [0:32] w1tp ch0, [32:64] w1tp ch1,
    # [64:96] band-summing selector SumP[32*m + h, h] = 1, [96] fc1
    # bias, [128:1152] w2r rows 0..32 (1/HW folded into the relu scale)
    cpk = np.zeros((128, 128 + 8 * 128), np.float32)
    w1t = fc1_w.T.astype(np.float32)  # [256, 32]
    cpk[:, 0:HID] = w1t[0:128]
    cpk[:, HID:2 * HID] = w1t[128:256]
    cpk[:, 2 * HID:3 * HID] = np.tile(np.eye(HID, dtype=np.float32), (4, 1))
    cpk[0:HID, 3 * HID] = fc1_b.astype(np.float32)
    # fc2 as [HID+1, 1024] with col o = j*128 + c, j = k*4 + isbeta*2 + ch;
    # row HID carries fc2_b (ones-row trick)
    for k in range(2):
        for isbeta in range(2):
            wab = fc2_w[k, isbeta::2, :].astype(np.float32)  # [256, 32]
            bab = fc2_b[k, isbeta::2].astype(np.float32)     # [256]
            for ch in range(2):
                j = k * 4 + isbeta * 2 + ch
                sl = slice(128 + j * 128, 128 + (j + 1) * 128)
                cpk[:HID, sl] = wab[128 * ch:128 * (ch + 1), :].T
                cpk[HID, sl] = bab[128 * ch:128 * (ch + 1)]
    cpk = cpk.astype(bf16)
    in_maps = []
    for i in range(NCORES):
        in_maps.append({
            "xs": np.ascontiguousarray(x[NPC * i:NPC * (i + 1)]),
            "cpk": cpk,
        })
    return in_maps


def kernel(x, fc1_w, fc1_b, fc2_w, fc2_b):
    from concourse.bass_utils import run_bass_kernel_spmd

    nc = _build_program()
    in_maps = make_inputs(x, fc1_w, fc1_b, fc2_w, fc2_b)
    res = run_bass_kernel_spmd(nc, in_maps, core_ids=list(range(NCORES)))
    shards = [
        np.asarray(res.results[i]["out"]).astype(np.float32)
        for i in range(NCORES)
    ]
    return np.concatenate(shards, axis=0).reshape(N, C, H, W)


if __name__ == "__main__":
    rng = np.random.default_rng(0)
    x = rng.standard_normal((N, C, H, W), dtype=np.float32)
    fc1_w = rng.standard_normal((HID, C), dtype=np.float32) * 0.06
    fc1_b = rng.standard_normal((HID,), dtype=np.float32) * 0.06
    fc2_w = rng.standard_normal((2, 2 * C, HID), dtype=np.float32) * 0.17
    fc2_b = rng.standard_normal((2, 2 * C), dtype=np.float32) * 0.17
    out = kernel(x, fc1_w, fc1_b, fc2_w, fc2_b)
    print(out.shape, out.dtype)


# revision 43
# speedup vs baseline: 1.1687x; 1.1687x over previous
"""Trainium2 Bass kernel for Dynamic ReLU-B (nn_Dynamic_Relu_B_70291434766473).

Reference computation (per sample n, channel c, pixel p):
    pooled[n,c] = mean_p x[n,c,p]
    h = relu(pooled @ fc1_w.T + fc1_b)                       # [N, 32]
    delta = 2*sigmoid(einsum('koh,nh->kno', fc2_w, h) + fc2_b) - 1
    alpha = delta[..., 0::2]; beta = delta[..., 1::2]        # [K, N, C]
    a = [1,0][k] + 1.0*alpha ; b = [1,0][k] + 0.5*beta
    out = max_k (x * a[k] + b[k])

Strategy: pure data parallel over batch N=32 across 8 NeuronCores (4
samples/core).  x and out move through HBM as bf16 (host casts), which
halves DMA traffic to ~12.9 MB/core and unlocks the DVE 16-bit perf
modes (4x tensor_scalar, 2x tensor_tensor).  End-to-end bf16 error is
~5e-3 vs the 2e-2 gate.

  - pooling runs on the otherwise-idle PE via linearity:
    fc1_pre = W1 @ (sum_p x) = sum_p (W1 @ x).  Per sample, 8 pixel
    chunks of 392 go through matmuls with the [128, 32] fc1 weights,
    landing in 4 partition-bands x 2 column-halves of one [128, 1024]
    (2-bank) PSUM tile; a single accumulate pass per column-half (one
    on ACT, one on DVE) gives band partials, and a tiny [128, 32]
    summing selector matmul folds the 4 bands and lands fc1-preact on
    partitions 0..31.  relu(scale=1/HW, bias=fc1b) -> ht column.
  - fc2 is computed transposed: the [33, 128] w2r chunks are the
    stationary operand and the tiny ht column the moving one, so
    z lands per-channel on partitions directly ([128, 8] PSUM);
    2*sigmoid(z)-1 = tanh(z/2) is one [128, 8] ACT op.  No PE
    transposes needed.
  - apply: branch k=1 on ScalarE activation(Identity, scale=a1,
    bias=b1) in pixel-halves (keeps the ACT queue responsive for the
    tiny head ops), late tiles on VectorE; branch k=0 and the max on
    VectorE (bf16 4x / 2x modes); params stay fp32 per-partition
    scalars (exempt from the 16-bit perf-mode dtype rule).
  - DMA: x loads ride ch0->SP ring, ch1->ACT ring (parallel ramp),
    all stores ride the SP ring behind the ch0 loads; the last
    sample's stores split across both rings.  All constants ride ONE
    packed SWDGE (gpsimd) transfer (SWDGE serializes at ~2us/DMA).
    First two samples load in pixel-halves, last two apply in
    pixel-halves (shorter ramp/tail).  Order-only deps keep the tiny
    per-sample head ops (presum/zT on PE, tanh/ab on ACT) ahead of
    streaming ops in each engine's static queue — the Tile scheduler's
    optimistic DMA model otherwise front-loads the next sample's G
    matmuls, which stall on real loads and head-of-line block the
    previous head.
"""

import numpy as np

N, C, H, W = 32, 256, 56, 56
HW = H * W
HID = C // 8  # 32
NCORES = 8
NPC = N // NCORES  # samples per core
NCH = 8            # pixel chunks for the PE pooling matmuls
CHK = HW // NCH    # 392

_CACHE = {}


def _build_program():
    """Build (and cache) the compiled Bass program for one core."""
    if "nc" in _CACHE:
        return _CACHE["nc"]

    import concourse.bacc as bacc
    import concourse.mybir as mybir
    import concourse.tile as tile

    f32 = mybir.dt.float32
    bf16 = mybir.dt.bfloat16
    AF = mybir.ActivationFunctionType
    ALU = mybir.AluOpType

    nc = bacc.Bacc(
        "TRN2",
        target_bir_lowering=False,
        debug=False,
        enable_asserts=True,
        num_devices=NCORES,
    )

    xs = nc.dram_tensor("xs", [NPC, C, HW], bf16, kind="ExternalInput").ap()
    # all constants packed into ONE bf16 tensor: cols [0:32] w1tp ch0,
    # [32:64] w1tp ch1, [64:96] sump, [96] fc1b, [128:1152] w2r
    # (rows 0..32).  One SWDGE DMA instead of five — SWDGE serializes
    # at ~2us per transfer, which starved the first head in v1.
    cpk = nc.dram_tensor("cpk", [128, 128 + 8 * 128], bf16,
                         kind="ExternalInput").ap()
    out = nc.dram_tensor("out", [NPC, C, HW], bf16, kind="ExternalOutput").ap()

    ring = {0: nc.sync, 1: nc.scalar}  # per-channel-half HWDGE ring

    HH = HW // 2
    SPLIT_LOAD = (0, 1, 2)  # samples loaded in pixel-halves
    SPLIT_APPLY = (2, 3)  # samples whose apply runs in pixel-halves
    # y1 tiles on ACT (rest on DVE); sample 3 splits across engines.
    # (Pool/gpsimd TensorTensor and TensorScalar are rejected by walrus
    # codegen — "Instruction engine check failed (Pool)" — so the Pool
    # engine can only do memset here.)
    ACT_Y1 = {(0, 0), (0, 1), (1, 0), (1, 1), (3, 0)}
    POOL_MAX = set()

    with tile.TileContext(nc) as tc:
        with (
            tc.tile_pool(name="const", bufs=1) as cpool,
            tc.tile_pool(name="x", bufs=2 * NPC) as xpool,
            tc.tile_pool(name="y1", bufs=4) as ypool,
            tc.tile_pool(name="o", bufs=6) as opool,
            tc.tile_pool(name="rd", bufs=2) as rpool,
            tc.tile_pool(name="small", bufs=1) as smpool,
            tc.tile_pool(name="ps", bufs=2, space="PSUM") as pspool,
        ):
            # --- constants: one packed SWDGE transfer off the HWDGE rings ---
            cpk_t = cpool.tile([128, 128 + 8 * 128], bf16, tag="cpk")
            nc.gpsimd.dma_start(cpk_t[:], cpk[:])
            w1tp_t = [cpk_t[:, 0:HID], cpk_t[:, HID:2 * HID]]
            sump_t = cpk_t[:, 2 * HID:3 * HID]
            w2r_t = cpk_t[0:HID + 1, 128:128 + 8 * 128]
            # fc1 bias needs f32 to pair with the f32 psum input of the
            # relu; one-time tiny cast on DVE
            fc1b_t = cpool.tile([HID, 1], f32, tag="fc1b")
            nc.vector.tensor_copy(fc1b_t[:], cpk_t[0:HID, 3 * HID:3 * HID + 1])

            # ht holds all samples' hidden vectors; row HID is the ones row
            # for the fc2 bias trick (memset on the idle Pool engine).
            ht = smpool.tile([HID + 1, NPC], bf16, tag="ht")
            nc.gpsimd.memset(ht[HID:HID + 1, :], 1.0)

            # --- load all x tiles (ch0 -> sync ring, ch1 -> scalar ring) ---
            xt = {}
            last_disp = {0: None, 1: None}
            for n in range(NPC):
                for ch in range(2):
                    t = xpool.tile([128, HW], bf16, tag="x")
                    if n in SPLIT_LOAD:
                        for h in range(2):
                            last_disp[ch] = ring[ch].dma_start(
                                t[:, h * HH:(h + 1) * HH],
                                xs[n, ch * 128:(ch + 1) * 128,
                                   h * HH:(h + 1) * HH],
                            )
                    else:
                        last_disp[ch] = ring[ch].dma_start(
                            t[:], xs[n, ch * 128:(ch + 1) * 128, :]
                        )
                    xt[(n, ch)] = t

            tts, abs_, tanh_insts = {}, {}, {}
            ab_last, rb_insts, zt_last = {}, {}, {}

            def head(s):
                # G = W1 @ x via PE: chunk j -> band j%4, col-half j//4.
                # [128, 1024] = exactly 2 PSUM banks; col-half 1 starts at
                # col 512 so no matmul crosses a bank boundary.
                g = pspool.tile([128, 1024], f32, tag="g")
                for j in range(NCH):
                    band = j % 4
                    col = (j // 4) * 512
                    for ch in range(2):
                        mm = nc.tensor.matmul(
                            g[32 * band:32 * band + 32, col:col + CHK],
                            w1tp_t[ch],
                            xt[(s, ch)][:, j * CHK:(j + 1) * CHK],
                            start=(ch == 0), stop=(ch == 1),
                            tile_position=(0, 32 * band),
                        )
                        if j == 0 and ch == 0 and s - 1 in zt_last:
                            # the scheduler's DMA model is optimistic: it
                            # front-loads G(s) into the PE queue where it
                            # stalls on real loads, head-of-line blocking
                            # the previous head's tiny presum/zT matmuls.
                            tile.add_dep_helper(
                                mm.ins, zt_last[s - 1].ins, sync=False,
                                reason="prev head zT before next G",
                            )
                # reduce each column-half over pixels -> band partials
                # (bf16 partials so they can feed the bf16 selector matmul)
                pre2 = smpool.tile([128, 2], bf16, tag=f"pre{s}")
                dA = rpool.tile([128, CHK], f32, tag="dA")
                dB = rpool.tile([128, CHK], f32, tag="dB")
                with nc.allow_low_precision(reason="bf16 band partials"):
                    ra = nc.scalar.activation(
                        dA[:], g[:, 0:CHK], AF.Copy, accum_out=pre2[:, 0:1],
                    )
                    if s >= 2:
                        # late samples: the DVE queue is backed up with
                        # apply ops by now, so a DVE-side reduce would
                        # delay presum/relu (observed 2.1-2.6us ACT idle);
                        # run it on ACT right behind rA instead
                        nc.scalar.activation(
                            dB[:], g[:, 512:512 + CHK], AF.Copy,
                            accum_out=pre2[:, 1:2],
                        )
                    else:
                        rb_insts[s] = nc.vector.tensor_scalar(
                            dB[:], g[:, 512:512 + CHK], 1.0, None,
                            ALU.mult, ALU.add, accum_out=pre2[:, 1:2],
                        )
                if s == 0:
                    # all ch1 load dispatches (which share the ACT engine
                    # queue) must come before any ACT compute
                    tile.add_dep_helper(
                        ra.ins, last_disp[1].ins, sync=False,
                        reason="load dispatches before ACT compute",
                    )
                # band-sum + partition shift onto 0..31 via selector matmul
                php = pspool.tile([HID, 1], f32, tag="php")
                nc.tensor.matmul(
                    php[:], sump_t, pre2[:, 0:1], start=True, stop=False,
                )
                nc.tensor.matmul(
                    php[:], sump_t, pre2[:, 1:2], start=False, stop=True,
                )
                # fc1: relu(php/HW + fc1b) -> ht column s
                nc.scalar.activation(
                    ht[0:HID, s:s + 1], php[:],
                    AF.Relu, bias=fc1b_t[:], scale=1.0 / HW,
                )
                # fc2 transposed: z lands per-channel on partitions
                pzt = pspool.tile([128, 8], f32, tag="pzt")
                for cb in range(8):
                    zt_last[s] = nc.tensor.matmul(
                        pzt[:, cb:cb + 1],
                        cpk_t[0:HID + 1, 128 + cb * 128:128 + (cb + 1) * 128],
                        ht[:, s:s + 1],
                        start=True, stop=True,
                    )
                # t = tanh(z/2) = 2*sigmoid(z) - 1
                tt = smpool.tile([128, 8], f32, tag=f"tt{s}")
                tanh_insts[s] = nc.scalar.activation(
                    tt[:], pzt[:], AF.Tanh, bias=0.0, scale=0.5
                )
                tts[s] = tt
                # param transforms on ACT right behind the tanh (tiny; on
                # DVE they head-of-line block behind streaming apply ops):
                #   a0 = 1 + t   b0 = 0.5*t + 1   a1 = t   b1 = 0.5*t
                ab = smpool.tile([128, 8], f32, tag=f"ab{s}")
                nc.scalar.activation(
                    ab[:, 0:2], tt[:, 0:2], AF.Identity, bias=1.0, scale=1.0
                )
                nc.scalar.activation(
                    ab[:, 2:4], tt[:, 2:4], AF.Identity, bias=1.0, scale=0.5
                )
                ab_last[s] = nc.scalar.activation(
                    ab[:, 6:8], tt[:, 6:8], AF.Identity, bias=0.0, scale=0.5
                )
                abs_[s] = ab

            def apply_s(s):
                # emit all y1s, then all y0s, then the maxes, so neither
                # engine's queue head-of-line blocks on the other engine.
                tt, ab = tts[s], abs_[s]
                units = []  # (ch, pixel-slice) at store granularity
                for ch in range(2):
                    if s in SPLIT_APPLY:
                        units.append((ch, slice(0, HH)))
                        units.append((ch, slice(HH, HW)))
                    else:
                        units.append((ch, slice(0, HW)))
                y1s, os_ = {}, {}
                for u, (ch, sl) in enumerate(units):
                    y1 = ypool.tile([128, sl.stop - sl.start], bf16, tag="y1")
                    if (s, ch) in ACT_Y1:
                        # halves keep the ACT queue responsive for head ops
                        m = (sl.stop - sl.start) // 2
                        for hs in (slice(0, m), slice(m, sl.stop - sl.start)):
                            inst = nc.scalar.activation(
                                y1[:, hs],
                                xt[(s, ch)][:, sl.start + hs.start:
                                            sl.start + hs.stop],
                                AF.Identity,
                                bias=ab[:, 6 + ch:7 + ch],
                                scale=tt[:, 4 + ch:5 + ch],
                            )
                            if s + 1 in ab_last:
                                # keep the next head's tanh/ab chain ahead
                                # of these streaming ops on ACT
                                tile.add_dep_helper(
                                    inst.ins, ab_last[s + 1].ins,
                                    sync=False,
                                    reason="head chain before streaming y1",
                                )
                    else:
                        nc.vector.tensor_scalar(
                            y1[:], xt[(s, ch)][:, sl],
                            tt[:, 4 + ch:5 + ch], ab[:, 6 + ch:7 + ch],
                            ALU.mult, ALU.add,
                        )
                    y1s[u] = y1
                for u, (ch, sl) in enumerate(units):
                    o = opool.tile([128, sl.stop - sl.start], bf16, tag="o")
                    nc.vector.tensor_scalar(
                        o[:], xt[(s, ch)][:, sl],
                        ab[:, ch:ch + 1], ab[:, 2 + ch:3 + ch],
                        ALU.mult, ALU.add,
                    )
                    os_[u] = o
                for u, (ch, sl) in enumerate(units):
                    o, y1 = os_[u], y1s[u]
                    if (s, ch) in POOL_MAX:
                        nc.gpsimd.tensor_tensor(o[:], o[:], y1[:], ALU.max)
                    else:
                        nc.vector.tensor_max(o[:], o[:], y1[:])
                    if s == NPC - 1:
                        # final sample: split each store across both rings
                        # so the tail transfers drain in parallel
                        m = (sl.stop - sl.start) // 2
                        ring[0].dma_start(
                            out[s, ch * 128:(ch + 1) * 128,
                                sl.start:sl.start + m], o[:, 0:m],
                        )
                        ring[1].dma_start(
                            out[s, ch * 128:(ch + 1) * 128,
                                sl.start + m:sl.stop], o[:, m:],
                        )
                    else:
                        ring[0].dma_start(
                            out[s, ch * 128:(ch + 1) * 128, sl], o[:]
                        )

            # heads lead (they pace on DMA arrival); each sample's apply
            # streams behind the NEXT head so param latency never hides
            # behind streaming ops.
            head(0)
            head(1)
            apply_s(0)
            head(2)
            apply_s(1)
            head(3)
            apply_s(2)
            apply_s(3)

    nc.compile()
    _CACHE["nc"] = nc
    return nc


def make_inputs(x, fc1_w, fc1_b, fc2_w, fc2_b):
    """Host-side prep: shard x, rearrange weights into device layouts."""
    import ml_dtypes

    bf16 = ml_dtypes.bfloat16
    x = np.ascontiguousarray(
        np.asarray(x, dtype=np.float32).reshape(N, C, HW)
    ).astype(bf16)
    # one packed bf16 const tensor: [0:32] w1tp ch0, [32:64] w1tp ch1,
    # [64:96] band-summing selector SumP[32*m + h, h] = 1, [96] fc1
    # bias, [128:1152] w2r rows 0..32 (1/HW folded into the relu scale)
    cpk = np.zeros((128, 128 + 8 * 128), np.float32)
    w1t = fc1_w.T.astype(np.float32)  # [256, 32]
    cpk[:, 0:HID] = w1t[0:128]
    cpk[:, HID:2 * HID] = w1t[128:256]
    cpk[:, 2 * HID:3 * HID] = np.tile(np.eye(HID, dtype=np.float32), (4, 1))
    cpk[0:HID, 3 * HID] = fc1_b.astype(np.float32)
    # fc2 as [HID+1, 1024] with col o = j*128 + c, j = k*4 + isbeta*2 + ch;
    # row HID carries fc2_b (ones-row trick)
    for k in range(2):
        for isbeta in range(2):
            wab = fc2_w[k, isbeta::2, :].astype(np.float32)  # [256, 32]
            bab = fc2_b[k, isbeta::2].astype(np.float32)     # [256]
            for ch in range(2):
                j = k * 4 + isbeta * 2 + ch
                sl = slice(128 + j * 128, 128 + (j + 1) * 128)
                cpk[:HID, sl] = wab[128 * ch:128 * (ch + 1), :].T
                cpk[HID, sl] = bab[128 * ch:128 * (ch + 1)]
    cpk = cpk.astype(bf16)
    in_maps = []
    for i in range(NCORES):
        in_maps.append({
            "xs": np.ascontiguousarray(x[NPC * i:NPC * (i + 1)]),
            "cpk": cpk,
        })
    return in_maps


def kernel(x, fc1_w, fc1_b, fc2_w, fc2_b):
    from concourse.bass_utils import run_bass_kernel_spmd

    nc = _build_program()
    in_maps = make_inputs(x, fc1_w, fc1_b, fc2_w, fc2_b)
    res = run_bass_kernel_spmd(nc, in_maps, core_ids=list(range(NCORES)))
    shards = [
        np.asarray(res.results[i]["out"]).astype(np.float32)
        for i in range(NCORES)
    ]
    return np.concatenate(shards, axis=0).reshape(N, C, H, W)


if __name__ == "__main__":
    rng = np.random.default_rng(0)
    x = rng.standard_normal((N, C, H, W), dtype=np.float32)
    fc1_w = rng.standard_normal((HID, C), dtype=np.float32) * 0.06
    fc1_b = rng.standard_normal((HID,), dtype=np.float32) * 0.06
    fc2_w = rng.standard_normal((2, 2 * C, HID), dtype=np.float32) * 0.17
    fc2_b = rng.standard_normal((2, 2 * C), dtype=np.float32) * 0.17
    out = kernel(x, fc1_w, fc1_b, fc2_w, fc2_b)
    print(out.shape, out.dtype)
